# revision 13
# baseline (speedup 1.0000x reference)
"""Expert-parallel grouped GEMM (MoE) kernel for Trainium2.

Problem: out[e] = gelu(tok[e] @ w1[e]) @ w2[e]  per expert e.
  tok: [128, 2048, 128] f32, w1: [128, 128, 512] f32, w2: [128, 512, 128] f32.

Sharding: expert-parallel across 8 NeuronCores, 16 experts per core, no
cross-core communication. Each core runs the same Bass program on its own
expert slice (SPMD), the host concatenates the per-core outputs.

v2 design (bf16 datapath):
  - host casts tok/w1/w2 to bf16 (graded rel-err gate 2e-2 >> bf16 ~2e-3)
  - tokens DMA-xbar-transposed straight into [d, t] layout (2-byte dtype
    unlocks the HWDGE transpose path; PE does no input transposes)
  - GEMM1: hT[hd] = w1_slice.T @ tokT, bf16 operands (FWL weight loads),
    fp32 PSUM; GELU (exact erf) on ScalarE -> bf16 SBUF
  - GEMM2 "mm_nat": token-block slices of hT are the STATIONARY operand,
    w2 k-tiles stream as the moving operand -> output is produced directly
    in natural [t, o] layout; no output transpose on any engine
  - GEMM2 "pe_t" (alt): w2 stationary, hT moving -> outT, then PE
    transpose back (the v1 tail)
  - f32 output stores: each token row is a contiguous 512 B piece
"""

import numpy as np

NUM_CORES = 8
E_TOTAL = 128
E_PER_CORE = E_TOTAL // NUM_CORES  # 16
T = 2048
D = 128
H = 512
O = 128
P = 128

T_CHUNK = 512
N_CHUNKS = T // T_CHUNK  # 4
BLKS = T_CHUNK // P  # 4
H_TILES = H // P  # 4

_CACHE = {}


DEFAULT_CFG = dict(
    out_mode="mm_nat",  # "mm_nat" | "pe_t"
    out_dt="f32",  # "f32" | "bf16"
    tokt_bufs=3,
    h_bufs=3,
    w_bufs=3,
    ph_bufs=2,
    po_bufs=2,
    outn_bufs=3,
    osb_bufs=2,
    pot_bufs=1,
    mm2_hd_outer=True,  # hd outer / j inner: MM2 can start after first GELU pair
)


def _build(loop=1, cfg=None):
    import concourse.bacc as bacc
    import concourse.mybir as mybir
    import concourse.tile as tile
    from concourse.masks import make_identity

    f32 = mybir.dt.float32
    bf16 = mybir.dt.bfloat16
    C = dict(DEFAULT_CFG)
    if cfg:
        C.update(cfg)
    # debug: sim has no Gelu; Tanh is elementwise too, keeps dataflow identical
    GELU = (
        mybir.ActivationFunctionType.Tanh
        if C.get("dbg_tanh")
        else mybir.ActivationFunctionType.Gelu
    )
    out_dt = f32 if C["out_dt"] == "f32" else bf16

    nc = bacc.Bacc(
        "TRN2",
        target_bir_lowering=False,
        debug=False,
        num_devices=NUM_CORES,
    )

    tok = nc.dram_tensor(
        "group_token", [E_PER_CORE, T, D], bf16, kind="ExternalInput"
    ).ap()
    w1 = nc.dram_tensor("weights1", [E_PER_CORE, D, H], bf16, kind="ExternalInput").ap()
    w2 = nc.dram_tensor("weights2", [E_PER_CORE, H, O], bf16, kind="ExternalInput").ap()
    out = nc.dram_tensor("out", [E_PER_CORE, T, O], out_dt, kind="ExternalOutput").ap()

    with tile.TileContext(nc) as tc:
        with (
            tc.tile_pool(name="const", bufs=1) as const_pool,
            tc.tile_pool(name="weights", bufs=C["w_bufs"]) as w_pool,
            tc.tile_pool(name="tokt", bufs=C["tokt_bufs"]) as tokt_pool,
            tc.tile_pool(name="hts", bufs=C["h_bufs"]) as h_pool,
            tc.tile_pool(name="outn", bufs=C["outn_bufs"]) as outn_pool,
            tc.tile_pool(name="osb", bufs=C["osb_bufs"]) as osb_pool,
            tc.tile_pool(name="ph", bufs=C["ph_bufs"], space="PSUM") as ph_pool,
            tc.tile_pool(name="po", bufs=C["po_bufs"], space="PSUM") as po_pool,
            tc.tile_pool(name="pot", bufs=C["pot_bufs"], space="PSUM") as pot_pool,
        ):
            if C["out_mode"] == "pe_t":
                ident = const_pool.tile([P, P], bf16)
                idf = const_pool.tile([P, P], f32)
                make_identity(nc, idf)
                nc.vector.tensor_copy(ident[:], idf[:])

            NG = E_PER_CORE * N_CHUNKS  # 64 global chunks
            PF = C.get("prefetch_c", 2)  # chunk index at which next expert loads

            def body(_iv=None):
                # one-chunk software-pipelined emission: the PE stream is
                #   MM1(g) ; MM2(g-1) ; MM1(g+1) ; MM2(g) ; ...
                # so MM2(g-1)'s wait on GELU(g-1) overlaps MM1(g)'s streaming
                # instead of stalling the PE queue.
                state = {}  # e -> (tokt, w1_sb, w2_sb)
                hts = {}  # g -> ht tile

                def setup(e):
                    tokt = tokt_pool.tile([P, T], bf16, tag="tokt", name=f"tokt{e}")
                    nc.sync.dma_start(tokt[:], tok[e], transpose=True)
                    w1_sb = w_pool.tile([P, H], bf16, tag="w1", name=f"w1s{e}")
                    nc.gpsimd.dma_start(w1_sb[:], w1[e])
                    w2_sb = w_pool.tile([P, H_TILES, O], bf16, tag="w2", name=f"w2s{e}")
                    # weights2 is host-permuted so each partition's 4 k-tile
                    # rows are contiguous in DRAM: 128 descriptors, not 512
                    nc.gpsimd.dma_start(
                        w2_sb[:], w2[e].rearrange("(p k) o -> p k o", k=H_TILES)
                    )
                    state[e] = (tokt, w1_sb, w2_sb)

                phs = {}  # g -> ph tile (quad mode: MM2 reuses bank 0 as po)

                def mm1(g):
                    e, c = divmod(g, N_CHUNKS)
                    tokt, w1_sb, _ = state[e]
                    tslc = tokt[:, c * T_CHUNK : (c + 1) * T_CHUNK]
                    ht = h_pool.tile(
                        [P, H_TILES, T_CHUNK], bf16, tag="ht", name=f"ht{g}"
                    )
                    if C.get("gelu_quad", False):
                        # one 4-bank PSUM tile + one GELU per chunk; MM2 later
                        # reuses bank 0 of this tile as its accumulator (WAR
                        # dep on the GELU read keeps it safe), so 2 bufs fill
                        # all 8 PSUM banks with no separate po pool. Bank 0 is
                        # written LAST so its extra dep (the DVE drain of the
                        # previous use as accumulator) gates only one matmul.
                        ph = ph_pool.tile([P, H_TILES, T_CHUNK], f32, tag="phq")
                        for hd in (1, 2, 3, 0):
                            nc.tensor.matmul(
                                ph[:, hd],
                                w1_sb[:, hd * P : (hd + 1) * P],
                                tslc,
                                start=True,
                                stop=True,
                            )
                        nc.scalar.activation(ht[:], ph[:], GELU)
                        phs[g] = ph
                    elif C.get("gelu_single"):
                        for hd in range(H_TILES):
                            ph = ph_pool.tile([P, T_CHUNK], f32, tag="ph1")
                            nc.tensor.matmul(
                                ph[:],
                                w1_sb[:, hd * P : (hd + 1) * P],
                                tslc,
                                start=True,
                                stop=True,
                            )
                            nc.scalar.activation(ht[:, hd], ph[:], GELU)
                    else:
                        for hp in range(H_TILES // 2):
                            ph = ph_pool.tile([P, 2, T_CHUNK], f32, tag="ph")
                            for k in range(2):
                                hd = hp * 2 + k
                                nc.tensor.matmul(
                                    ph[:, k],
                                    w1_sb[:, hd * P : (hd + 1) * P],
                                    tslc,
                                    start=True,
                                    stop=True,
                                )
                            nc.scalar.activation(
                                ht[:, hp * 2 : hp * 2 + 2], ph[:], GELU
                            )
                    hts[g] = ht

                N_BLKS = N_CHUNKS * BLKS  # 16 token blocks per expert
                ocs = {}  # e -> per-expert output staging tile

                def mm2_and_store(g):
                    e, c = divmod(g, N_CHUNKS)
                    _, _, w2_sb = state[e]
                    ht = hts.pop(g)
                    if c == 0:
                        ocs[e] = outn_pool.tile(
                            [P, N_BLKS, O], out_dt, tag="oc", name=f"oc{e}"
                        )
                    oc = ocs[e]
                    if C["out_mode"] == "mm_nat":
                        if C.get("gelu_quad", False):
                            po = phs.pop(g)[:, 0].rearrange(
                                "p (j o) -> p j o", j=BLKS
                            )
                        else:
                            po_t = po_pool.tile([P, BLKS, O], f32, tag="po")
                            po = po_t[:]
                        for j in range(BLKS):
                            for hd in range(H_TILES):
                                nc.tensor.matmul(
                                    po[:, j],
                                    ht[:, hd, j * P : (j + 1) * P],
                                    w2_sb[:, hd],
                                    start=(hd == 0),
                                    stop=(hd == H_TILES - 1),
                                )
                        nc.vector.tensor_copy(
                            oc[:, c * BLKS : (c + 1) * BLKS], po
                        )
                    else:  # pe_t
                        po = po_pool.tile([P, T_CHUNK], f32, tag="po")
                        for hd in range(H_TILES):
                            nc.tensor.matmul(
                                po[:],
                                w2_sb[:, hd],
                                ht[:, hd],
                                start=(hd == 0),
                                stop=(hd == H_TILES - 1),
                            )
                        osb = osb_pool.tile([P, T_CHUNK], bf16, tag="osb")
                        nc.vector.tensor_copy(osb[:], po[:])
                        pot = pot_pool.tile([P, T_CHUNK], bf16, tag="pot")
                        for j in range(BLKS):
                            nc.tensor.transpose(
                                pot[:, j * P : (j + 1) * P],
                                osb[:, j * P : (j + 1) * P],
                                ident[:],
                            )
                        nc.vector.tensor_copy(
                            oc[:, c * BLKS : (c + 1) * BLKS],
                            pot[:].rearrange("p (j o) -> p j o", j=BLKS),
                        )
                    if c == N_CHUNKS - 1:
                        nc.sync.dma_start(
                            out[e].rearrange("(j p) o -> p j o", p=P),
                            ocs.pop(e)[:],
                        )

                setup(0)
                for g in range(NG):
                    e, c = divmod(g, N_CHUNKS)
                    if c == PF and e + 1 < E_PER_CORE:
                        setup(e + 1)
                    mm1(g)
                    if g >= 1:
                        mm2_and_store(g - 1)
                mm2_and_store(NG - 1)

            if loop == 1:
                body()
            else:
                with tc.For_i(0, loop, 1) as _i:
                    body(_i)

    nc.compile()
    return nc


def _get_nc(loop=1, cfg=None):
    key = ("nc", loop, tuple(sorted((cfg or {}).items())))
    if key not in _CACHE:
        _CACHE[key] = _build(loop, cfg)
    return _CACHE[key]


def permute_w2(w2_bf16):
    """Row-permute each expert's w2 so the device-side [p, k, o] SBUF load is
    contiguous per partition: host row (p*H_TILES + k) = original row (k*P + p).
    """
    e = w2_bf16.shape[0]
    return np.ascontiguousarray(
        w2_bf16.reshape(e, H_TILES, P, O).transpose(0, 2, 1, 3).reshape(e, H, O)
    )


def kernel(group_token, weights1, weights2):
    import ml_dtypes
    from concourse.bass_utils import run_bass_kernel_spmd

    bf16 = ml_dtypes.bfloat16
    group_token = np.asarray(group_token).astype(bf16)
    weights1 = np.asarray(weights1).astype(bf16)
    weights2 = permute_w2(np.asarray(weights2).astype(bf16))

    nc = _get_nc()
    in_maps = []
    for c in range(NUM_CORES):
        sl = slice(c * E_PER_CORE, (c + 1) * E_PER_CORE)
        in_maps.append(
            {
                "group_token": np.ascontiguousarray(group_token[sl]),
                "weights1": np.ascontiguousarray(weights1[sl]),
                "weights2": np.ascontiguousarray(weights2[sl]),
            }
        )

    res = run_bass_kernel_spmd(nc, in_maps, core_ids=list(range(NUM_CORES)))
    _CACHE["last_results"] = res
    full = np.concatenate([r["out"] for r in res.results], axis=0)
    return full.astype(np.float32)


# revision 15
# speedup vs baseline: 1.0426x; 1.0426x over previous
"""Expert-parallel grouped GEMM (MoE) kernel for Trainium2.

Problem: out[e] = gelu(tok[e] @ w1[e]) @ w2[e]  per expert e.
  tok: [128, 2048, 128] f32, w1: [128, 128, 512] f32, w2: [128, 512, 128] f32.

Sharding: expert-parallel across 8 NeuronCores, 16 experts per core, no
cross-core communication. Each core runs the same Bass program on its own
expert slice (SPMD), the host concatenates the per-core outputs.

v2 design (bf16 datapath):
  - host casts tok/w1/w2 to bf16 (graded rel-err gate 2e-2 >> bf16 ~2e-3)
  - tokens DMA-xbar-transposed straight into [d, t] layout (2-byte dtype
    unlocks the HWDGE transpose path; PE does no input transposes)
  - GEMM1: hT[hd] = w1_slice.T @ tokT, bf16 operands (FWL weight loads),
    fp32 PSUM; GELU (exact erf) on ScalarE -> bf16 SBUF
  - GEMM2 "mm_nat": token-block slices of hT are the STATIONARY operand,
    w2 k-tiles stream as the moving operand -> output is produced directly
    in natural [t, o] layout; no output transpose on any engine
  - GEMM2 "pe_t" (alt): w2 stationary, hT moving -> outT, then PE
    transpose back (the v1 tail)
  - f32 output stores: each token row is a contiguous 512 B piece
"""

import numpy as np

NUM_CORES = 8
E_TOTAL = 128
E_PER_CORE = E_TOTAL // NUM_CORES  # 16
T = 2048
D = 128
H = 512
O = 128
P = 128

T_CHUNK = 512
N_CHUNKS = T // T_CHUNK  # 4
BLKS = T_CHUNK // P  # 4
H_TILES = H // P  # 4

_CACHE = {}


DEFAULT_CFG = dict(
    out_mode="mm_nat",  # "mm_nat" | "pe_t"
    out_dt="f32",  # "f32" | "bf16"
    tokt_bufs=3,
    h_bufs=3,
    w_bufs=3,
    ph_bufs=2,
    po_bufs=2,
    outn_bufs=3,
    osb_bufs=3,
    pot_bufs=2,
    mm2_hd_outer=True,  # hd outer / j inner: MM2 can start after first GELU pair
)


def _build(loop=1, cfg=None):
    import concourse.bacc as bacc
    import concourse.mybir as mybir
    import concourse.tile as tile
    from concourse.masks import make_identity

    f32 = mybir.dt.float32
    bf16 = mybir.dt.bfloat16
    C = dict(DEFAULT_CFG)
    if cfg:
        C.update(cfg)
    # debug: sim has no Gelu; Tanh is elementwise too, keeps dataflow identical
    GELU = (
        mybir.ActivationFunctionType.Tanh
        if C.get("dbg_tanh")
        else mybir.ActivationFunctionType.Gelu
    )
    out_dt = f32 if C["out_dt"] == "f32" else bf16

    nc = bacc.Bacc(
        "TRN2",
        target_bir_lowering=False,
        debug=False,
        num_devices=NUM_CORES,
    )

    tok = nc.dram_tensor(
        "group_token", [E_PER_CORE, T, D], bf16, kind="ExternalInput"
    ).ap()
    w1 = nc.dram_tensor("weights1", [E_PER_CORE, D, H], bf16, kind="ExternalInput").ap()
    w2 = nc.dram_tensor("weights2", [E_PER_CORE, H, O], bf16, kind="ExternalInput").ap()
    out = nc.dram_tensor("out", [E_PER_CORE, T, O], out_dt, kind="ExternalOutput").ap()

    with tile.TileContext(nc) as tc:
        with (
            tc.tile_pool(name="const", bufs=1) as const_pool,
            tc.tile_pool(name="weights", bufs=C["w_bufs"]) as w_pool,
            tc.tile_pool(name="tokt", bufs=C["tokt_bufs"]) as tokt_pool,
            tc.tile_pool(name="hts", bufs=C["h_bufs"]) as h_pool,
            tc.tile_pool(name="outn", bufs=C["outn_bufs"]) as outn_pool,
            tc.tile_pool(name="osb", bufs=C["osb_bufs"]) as osb_pool,
            tc.tile_pool(name="ph", bufs=C["ph_bufs"], space="PSUM") as ph_pool,
            tc.tile_pool(name="po", bufs=C["po_bufs"], space="PSUM") as po_pool,
            tc.tile_pool(name="pot", bufs=C["pot_bufs"], space="PSUM") as pot_pool,
        ):
            if C["out_mode"] == "pe_t":
                ident = const_pool.tile([P, P], bf16)
                idf = const_pool.tile([P, P], f32)
                make_identity(nc, idf)
                nc.vector.tensor_copy(ident[:], idf[:])

            NG = E_PER_CORE * N_CHUNKS  # 64 global chunks
            PF = C.get("prefetch_c", 2)  # chunk index at which next expert loads

            def body(_iv=None):
                # one-chunk software-pipelined emission: the PE stream is
                #   MM1(g) ; MM2(g-1) ; MM1(g+1) ; MM2(g) ; ...
                # so MM2(g-1)'s wait on GELU(g-1) overlaps MM1(g)'s streaming
                # instead of stalling the PE queue.
                state = {}  # e -> (tokt, w1_sb, w2_sb)
                hts = {}  # g -> ht tile

                def setup(e):
                    tokt = tokt_pool.tile([P, T], bf16, tag="tokt", name=f"tokt{e}")
                    nc.sync.dma_start(tokt[:], tok[e], transpose=True)
                    w1_sb = w_pool.tile([P, H], bf16, tag="w1", name=f"w1s{e}")
                    nc.gpsimd.dma_start(w1_sb[:], w1[e])
                    w2_sb = w_pool.tile([P, H_TILES, O], bf16, tag="w2", name=f"w2s{e}")
                    # weights2 is host-permuted so each partition's 4 k-tile
                    # rows are contiguous in DRAM: 128 descriptors, not 512
                    nc.gpsimd.dma_start(
                        w2_sb[:], w2[e].rearrange("(p k) o -> p k o", k=H_TILES)
                    )
                    state[e] = (tokt, w1_sb, w2_sb)

                phs = {}  # g -> ph tile (quad mode: MM2 reuses bank 0 as po)

                def mm1(g):
                    e, c = divmod(g, N_CHUNKS)
                    tokt, w1_sb, _ = state[e]
                    tslc = tokt[:, c * T_CHUNK : (c + 1) * T_CHUNK]
                    ht = h_pool.tile(
                        [P, H_TILES, T_CHUNK], bf16, tag="ht", name=f"ht{g}"
                    )
                    if C.get("gelu_quad", False):
                        # one 4-bank PSUM tile + one GELU per chunk; MM2 later
                        # reuses bank 0 of this tile as its accumulator (WAR
                        # dep on the GELU read keeps it safe), so 2 bufs fill
                        # all 8 PSUM banks with no separate po pool. Bank 0 is
                        # written LAST so its extra dep (the DVE drain of the
                        # previous use as accumulator) gates only one matmul.
                        ph = ph_pool.tile([P, H_TILES, T_CHUNK], f32, tag="phq")
                        for hd in (1, 2, 3, 0):
                            nc.tensor.matmul(
                                ph[:, hd],
                                w1_sb[:, hd * P : (hd + 1) * P],
                                tslc,
                                start=True,
                                stop=True,
                            )
                        nc.scalar.activation(ht[:], ph[:], GELU)
                        phs[g] = ph
                    elif C.get("gelu_single"):
                        for hd in range(H_TILES):
                            ph = ph_pool.tile([P, T_CHUNK], f32, tag="ph1")
                            nc.tensor.matmul(
                                ph[:],
                                w1_sb[:, hd * P : (hd + 1) * P],
                                tslc,
                                start=True,
                                stop=True,
                            )
                            nc.scalar.activation(ht[:, hd], ph[:], GELU)
                    else:
                        for hp in range(H_TILES // 2):
                            ph = ph_pool.tile([P, 2, T_CHUNK], f32, tag="ph")
                            for k in range(2):
                                hd = hp * 2 + k
                                nc.tensor.matmul(
                                    ph[:, k],
                                    w1_sb[:, hd * P : (hd + 1) * P],
                                    tslc,
                                    start=True,
                                    stop=True,
                                )
                            nc.scalar.activation(
                                ht[:, hp * 2 : hp * 2 + 2], ph[:], GELU
                            )
                    hts[g] = ht

                N_BLKS = N_CHUNKS * BLKS  # 16 token blocks per expert
                ocs = {}  # e -> per-expert output staging tile

                def mm2_and_store(g):
                    e, c = divmod(g, N_CHUNKS)
                    _, _, w2_sb = state[e]
                    ht = hts.pop(g)
                    if c == 0:
                        ocs[e] = outn_pool.tile(
                            [P, N_BLKS, O], out_dt, tag="oc", name=f"oc{e}"
                        )
                    oc = ocs[e]
                    if C["out_mode"] == "mm_nat":
                        if C.get("gelu_quad", False):
                            po = phs.pop(g)[:, 0].rearrange(
                                "p (j o) -> p j o", j=BLKS
                            )
                        else:
                            po_t = po_pool.tile([P, BLKS, O], f32, tag="po")
                            po = po_t[:]
                        for j in range(BLKS):
                            for hd in range(H_TILES):
                                nc.tensor.matmul(
                                    po[:, j],
                                    ht[:, hd, j * P : (j + 1) * P],
                                    w2_sb[:, hd],
                                    start=(hd == 0),
                                    stop=(hd == H_TILES - 1),
                                )
                        nc.vector.tensor_copy(
                            oc[:, c * BLKS : (c + 1) * BLKS], po
                        )
                    else:  # pe_t: MM2 + PSUM drain only; transposes go in xpose()
                        po = po_pool.tile([P, T_CHUNK], f32, tag="po")
                        for hd in range(H_TILES):
                            nc.tensor.matmul(
                                po[:],
                                w2_sb[:, hd],
                                ht[:, hd],
                                start=(hd == 0),
                                stop=(hd == H_TILES - 1),
                            )
                        osb = osb_pool.tile([P, T_CHUNK], bf16, tag="osb")
                        nc.vector.tensor_copy(osb[:], po[:])
                        osbs[g] = osb
                    if C["out_mode"] == "mm_nat" and c == N_CHUNKS - 1:
                        nc.sync.dma_start(
                            out[e].rearrange("(j p) o -> p j o", p=P),
                            ocs.pop(e)[:],
                        )

                osbs = {}

                def xpose(g):
                    # pe_t tail, skewed one further chunk behind MM2 so the
                    # transposes never wait on the osb DVE drain in-queue
                    e, c = divmod(g, N_CHUNKS)
                    osb = osbs.pop(g)
                    oc = ocs[e]
                    pot = pot_pool.tile([P, T_CHUNK], bf16, tag="pot")
                    for j in range(BLKS):
                        nc.tensor.transpose(
                            pot[:, j * P : (j + 1) * P],
                            osb[:, j * P : (j + 1) * P],
                            ident[:],
                        )
                    nc.vector.tensor_copy(
                        oc[:, c * BLKS : (c + 1) * BLKS],
                        pot[:].rearrange("p (j o) -> p j o", j=BLKS),
                    )
                    if c == N_CHUNKS - 1:
                        nc.sync.dma_start(
                            out[e].rearrange("(j p) o -> p j o", p=P),
                            ocs.pop(e)[:],
                        )

                pe_t = C["out_mode"] == "pe_t"
                setup(0)
                for g in range(NG):
                    e, c = divmod(g, N_CHUNKS)
                    if c == PF and e + 1 < E_PER_CORE:
                        setup(e + 1)
                    mm1(g)
                    if g >= 1:
                        mm2_and_store(g - 1)
                    if pe_t and g >= 2:
                        xpose(g - 2)
                mm2_and_store(NG - 1)
                if pe_t:
                    xpose(NG - 2)
                    xpose(NG - 1)

            if loop == 1:
                body()
            else:
                with tc.For_i(0, loop, 1) as _i:
                    body(_i)

    nc.compile()
    return nc


def _get_nc(loop=1, cfg=None):
    key = ("nc", loop, tuple(sorted((cfg or {}).items())))
    if key not in _CACHE:
        _CACHE[key] = _build(loop, cfg)
    return _CACHE[key]


def permute_w2(w2_bf16):
    """Row-permute each expert's w2 so the device-side [p, k, o] SBUF load is
    contiguous per partition: host row (p*H_TILES + k) = original row (k*P + p).
    """
    e = w2_bf16.shape[0]
    return np.ascontiguousarray(
        w2_bf16.reshape(e, H_TILES, P, O).transpose(0, 2, 1, 3).reshape(e, H, O)
    )


def kernel(group_token, weights1, weights2):
    import ml_dtypes
    from concourse.bass_utils import run_bass_kernel_spmd

    bf16 = ml_dtypes.bfloat16
    group_token = np.asarray(group_token).astype(bf16)
    weights1 = np.asarray(weights1).astype(bf16)
    weights2 = permute_w2(np.asarray(weights2).astype(bf16))

    nc = _get_nc()
    in_maps = []
    for c in range(NUM_CORES):
        sl = slice(c * E_PER_CORE, (c + 1) * E_PER_CORE)
        in_maps.append(
            {
                "group_token": np.ascontiguousarray(group_token[sl]),
                "weights1": np.ascontiguousarray(weights1[sl]),
                "weights2": np.ascontiguousarray(weights2[sl]),
            }
        )

    res = run_bass_kernel_spmd(nc, in_maps, core_ids=list(range(NUM_CORES)))
    _CACHE["last_results"] = res
    full = np.concatenate([r["out"] for r in res.results], axis=0)
    return full.astype(np.float32)
